# revision 3
# baseline (speedup 1.0000x reference)
"""ClothLinearFusion Trainium2 kernel: bf16 + PE k-reduction + PE clock
ramp grooming.

out[b, i] = (sum_k cloth[b, k, i]) * (sum_j f[i, j] * body[b, j])

Timeline-sim figures (concourse.timeline_sim, cost-model; tracked the
fp32 baseline at 22561 sim vs 29539 ns measured): this kernel sims at
13705 ns. HW rel err 0.0018 vs the 2e-2 gate (bf16 input quantization;
all accumulation fp32). Structure, driven by the sim trace:
- chunk 0 (carrying the 128x128 bf16 identity) moved to the FRONT of the
  single SP HWDGE ring (the ACT-ring trick serialized AFTER chunk 1 on the
  shared DMA engines and starved the first matmul of its stationary).
- all c_sum matmuls are 128-wide and accumulate into ONE (128,128) PSUM
  block: same total PE cycles as 512-wide groups, but the tail needs no
  DVE fold — the final elementwise mul reads c_sum and fv straight from
  PSUM right after the last matmul.
- the PE clock p-state ramps with sustained use (full speed only after
  ~3us of continuous execution, and any idle gap drops it back). Warmup
  filler matmuls on a memset scratch region start as soon as the SP ring
  is up, and small filler bursts pad the inter-chunk sem gaps so the PE
  never goes idle mid-stream: every data matmul after the ramp runs at
  full clock. Fillers read zeros and write a scratch PSUM bank nothing
  reads; counts are sim-tuned.
"""

import sys

sys.path.insert(0, "/opt/trn_rl_repo")

import numpy as np
import ml_dtypes

import bass_rust
import concourse.bass as bass
import concourse.mybir as mybir
import concourse.tile as tile
from concourse.bass_utils import run_bass_kernel_spmd
from concourse.vector_clock import ScopedClock

B = 128
K = 64
C = 1024
J = 1024
NCORES = 8
CI = C // NCORES
# chunk 0 carries the identity; chunks 1-2 carry the fv operands (4
# j-chunks each) so fv finishes mid-stream; the tail chunks taper
# geometrically (ratio ~91/53) so each chunk's matmuls retire just as the
# next chunk's completion sem fires.
KCHUNKS = [6, 14, 16, 13, 9, 6]
NBF = 2                           # chunks 1..NBF carry bf pieces
BFW = 1024                        # 4 j-chunks x (bodyT 128 | fT 128)
IDW = 128
# Filler matmuls (128-wide, on scratch) keeping the PE clock ramped:
# FILLERS[0] = warmup before chunk 0's matmuls, FILLERS[q+1] = pad after
# chunk q's matmuls. Sim-tuned.
FILLERS = [22, 24, 23, 2, 2, 2, 0]

F32 = mybir.dt.float32
BF16 = mybir.dt.bfloat16
NPBF16 = ml_dtypes.bfloat16

_CACHE = {}


# ---------------------------------------------------------------------------
# Framework patches (same as baseline): single-wait walrus + fast teardown.
# ---------------------------------------------------------------------------

def _split_drain_and_barrier(self, tick_clock, wait_clock):
    nc = self.nc
    drain_inst = nc.sync.drain()
    wait_clock.add_sem_waits(
        drain_inst.ins, ScopedClock({None: tick_clock.global_clock})
    )
    si = drain_inst.ins.sync_info
    if si is not None and len(si.on_wait) > 1:
        waits = list(si.on_wait)
        drain_inst.ins.sync_info = bass_rust.SyncInfo(
            on_wait=waits[:1], on_update=list(si.on_update)
        )
        for w in waits[1:]:
            extra = nc.sync.drain()
            extra.ins.sync_info = bass_rust.SyncInfo(on_wait=[w], on_update=[])

    nc.all_engine_barrier(sem_only=True)
    assert self.sems is not None
    popped = nc._tile_sem_poison_stack.pop()
    assert popped is self._sem_poison
    nc.clear_and_free_semaphores(list(self.sems.allocated().values()))
    nc.all_engine_barrier(sem_only=True)


tile.TileContext._drain_and_barrier = _split_drain_and_barrier


def _compact_to_ranges(nums):
    nums = sorted(set(nums))
    ranges = []
    start = prev = nums[0]
    for n in nums[1:]:
        if n == prev + 1:
            prev = n
            continue
        ranges.append(range(start, prev + 1))
        start = prev = n
    ranges.append(range(start, prev + 1))
    return ranges


def _fast_clear_and_free_semaphores(self, sems):
    if not sems:
        return
    sem_nums = [s.num if hasattr(s, "num") else s for s in sems]
    for sem_range in _compact_to_ranges(sem_nums):
        assert self._state.free_isdisjoint(sem_range)
        self.sync.drain(semaphore_range=sem_range)
        self.sync.sem_clear(sem_range)
    self._state.prepend_free_semaphores(sem_nums)
    for poison_set in self._tile_sem_poison_stack:
        poison_set.update(sem_nums)


def _strip_preamble(nc, keep_names=()):
    main_blk = None
    for fn in nc.m.functions:
        for blk in fn.blocks:
            if blk.name == "main":
                main_blk = blk
    assert main_blk is not None
    to_drop = []
    for inst in main_blk.instructions:
        t = type(inst).__name__
        if t == "InstMemset" and inst.name not in keep_names:
            to_drop.append(inst)
        elif t in ("InstDrain", "InstEventSemaphore"):
            to_drop.append(inst)
    for inst in to_drop:
        main_blk.instructions.remove(inst)


def _strip_same_engine_waits(nc):
    """Drop sem waits an instruction holds on its OWN engine's sem: the
    engines execute strictly in order, so a same-engine RAW is already
    ordered by the program; the walrus only accepts one wait anyway."""
    for fn in nc.m.functions:
        for blk in fn.blocks:
            for inst in blk.instructions:
                si = inst.sync_info
                if si is None or len(si.on_wait) <= 1:
                    continue
                eng = getattr(inst, "engine", None)
                ename = getattr(eng, "name", str(eng))
                keep, drop = [], []
                for w in si.on_wait:
                    prefix = str(w.ant_name).split("_")[0]
                    (drop if prefix == ename else keep).append(w)
                if drop and keep:
                    inst.sync_info = bass_rust.SyncInfo(
                        on_wait=keep, on_update=list(si.on_update)
                    )


def _assert_single_waits(nc):
    for fn in nc.m.functions:
        for blk in fn.blocks:
            for inst in blk.instructions:
                si = inst.sync_info
                if si is not None and len(si.on_wait) > 1:
                    raise AssertionError(
                        f"{type(inst).__name__} {inst.name} has "
                        f"{len(si.on_wait)} waits: "
                        f"{[(w.ant_name, w.wait_value) for w in si.on_wait]}"
                    )


# ---------------------------------------------------------------------------
# Kernel program (SPMD, identical on all 8 cores)
# ---------------------------------------------------------------------------

def _chunk_width(q, ks):
    w = ks * CI
    if q == 0:
        w += IDW
    elif 1 <= q <= NBF:
        w += BFW
    return w


def _build_program():
    nc = bass.Bass(target_bir_lowering=False, debug=False)
    nc.clear_and_free_semaphores = _fast_clear_and_free_semaphores.__get__(nc)

    ins = []
    for q, ks in enumerate(KCHUNKS):
        ins.append(
            nc.dram_tensor(f"in{q}", [B, _chunk_width(q, ks)], BF16,
                           kind="ExternalInput")
        )
    out = nc.dram_tensor("out_s", [B, CI], F32, kind="ExternalOutput")

    keep_memsets = []

    with tile.TileContext(nc) as tc:
        with (
            tc.tile_pool(name="pool", bufs=1) as pool,
            tc.tile_pool(name="psum", bufs=1, space=bass.MemorySpace.PSUM) as psum_pool,
        ):
            # --- single SP HWDGE ring, strict FIFO, chunk 0 first ---
            chunks = []
            for q, ks in enumerate(KCHUNKS):
                ch = pool.tile([B, _chunk_width(q, ks)], BF16, tag=f"ch{q}")
                nc.sync.dma_start(out=ch[:], in_=ins[q][:])
                chunks.append((ch, ks))

            ident = chunks[0][0][:, KCHUNKS[0] * CI: KCHUNKS[0] * CI + IDW]

            # --- PE warmup scratch: zeros matmul'd into a dead PSUM bank ---
            scratch = pool.tile([B, 2 * B], BF16, tag="scratch")
            ms = nc.vector.memset(scratch[:], 0.0)
            keep_memsets.append(ms.ins.name)
            scratch_psum = psum_pool.tile([B, B], F32)

            def filler(n):
                for _ in range(n):
                    nc.tensor.matmul(
                        scratch_psum[:],
                        scratch[:, 0:B],
                        scratch[:, B:2 * B],
                        start=True,
                        stop=True,
                    )

            # --- PE: c_sum (64 x 128-wide, one PSUM block) + fv ---
            csum_psum = psum_pool.tile([B, CI], F32)
            fv_psum = psum_pool.tile([B, CI], F32)
            fv_sb = pool.tile([B, CI], F32)
            n_csum = K
            n_fv = 4 * NBF
            ci_mm = 0
            fv_mm = 0
            filler(FILLERS[0])
            for q, (ch, ks) in enumerate(chunks):
                if 1 <= q <= NBF:
                    base = ks * CI
                    for h in range(4):
                        nc.tensor.matmul(
                            fv_psum[:],
                            ch[:, base + h * 256: base + h * 256 + B],
                            ch[:, base + h * 256 + B: base + h * 256 + B + CI],
                            start=(fv_mm == 0),
                            stop=(fv_mm == n_fv - 1),
                        )
                        fv_mm += 1
                for k in range(ks):
                    nc.tensor.matmul(
                        csum_psum[:],
                        ident,
                        ch[:, k * CI: (k + 1) * CI],
                        start=(ci_mm == 0),
                        stop=(ci_mm == n_csum - 1),
                    )
                    ci_mm += 1
                if q == NBF:
                    # fv complete: park it in SBUF while DVE is idle (the
                    # tail mul may read only ONE operand from PSUM)
                    nc.vector.tensor_copy(out=fv_sb[:], in_=fv_psum[:])
                filler(FILLERS[q + 1])

            # --- DVE tail: out = c_sum * fv straight from PSUM ---
            res = pool.tile([B, CI], F32)
            nc.vector.tensor_mul(out=res[:], in0=csum_psum[:], in1=fv_sb[:])
            nc.sync.dma_start(out=out[:], in_=res[:])

    _strip_preamble(nc, keep_names=keep_memsets)
    _strip_same_engine_waits(nc)
    _assert_single_waits(nc)
    return nc


def _get_program():
    if "nc" not in _CACHE:
        _CACHE["nc"] = _build_program()
    return _CACHE["nc"]


def _make_in_maps(cloth_latent, body_latent, f):
    cloth_latent = np.asarray(cloth_latent, dtype=np.float32)
    body_latent = np.asarray(body_latent, dtype=np.float32)
    f = np.asarray(f, dtype=np.float32)

    cloth_bf = cloth_latent.astype(NPBF16)            # (B, K, C)
    bodyT = body_latent.T.astype(NPBF16)              # (J, B)
    fT = f.T.astype(NPBF16)                           # (J, C)
    ident = np.eye(B, dtype=NPBF16)

    in_maps = []
    for i in range(NCORES):
        sl = slice(i * CI, (i + 1) * CI)
        bf = np.concatenate([bodyT, fT[:, sl]], axis=1)          # (J, 256)
        bf_r = bf.reshape(J // 128, 128, B + CI).transpose(1, 0, 2)
        cl = cloth_bf[:, :, sl]                                  # (B, K, CI)

        m = {}
        k0 = 0
        for q, ks in enumerate(KCHUNKS):
            cpart = cl[:, k0:k0 + ks, :].reshape(B, ks * CI)
            if q == 0:
                m[f"in{q}"] = np.ascontiguousarray(
                    np.concatenate([cpart, ident], axis=1)
                )
            elif 1 <= q <= NBF:
                bpart = bf_r[:, 4 * (q - 1): 4 * q, :].reshape(B, BFW)
                m[f"in{q}"] = np.ascontiguousarray(
                    np.concatenate([cpart, bpart], axis=1)
                )
            else:
                m[f"in{q}"] = np.ascontiguousarray(cpart)
            k0 += ks
        in_maps.append(m)
    return in_maps


def _run(cloth_latent, body_latent, f, trace=False):
    nc = _get_program()
    in_maps = _make_in_maps(cloth_latent, body_latent, f)
    r = run_bass_kernel_spmd(nc, in_maps, list(range(NCORES)), trace=trace)
    out = np.concatenate([r.results[i]["out_s"] for i in range(NCORES)], axis=1)
    return out, r


def kernel(cloth_latent, body_latent, f):
    out, _ = _run(cloth_latent, body_latent, f, trace=False)
    return out


def kernel_traced(cloth_latent, body_latent, f):
    """Returns (output, BassKernelResults) with NTFF profiling enabled."""
    return _run(cloth_latent, body_latent, f, trace=True)
